# revision 5
# baseline (speedup 1.0000x reference)
"""ALiBi causal attention on 8 Trainium2 NeuronCores.

Sharding: tensor-parallel over heads (2 heads/core) for QKV projection and
attention; AllToAll redistributes the (normalized, transposed) attention
outputs so each core owns a 512-token slice for the output projection.

Layout choices (all chosen to avoid on-chip transposes):
  - x is passed host-transposed as xT [D=1024, B*T=4096] in bf16.
  - Q/K are produced in "head-transposed" layout [head_dim, tokens] and
    augmented with one extra contraction row so that the per-query ALiBi
    term -slope*i rides the score matmul (exactly cancelled by softmax,
    so bf16 rounding of it is harmless).
  - Scores are computed transposed: ST[k, q] = (K'|aug).T-block @ (Q'|aug),
    so the softmax reduction (over k) aligns with the AV matmul contraction
    and the denominator falls out of a ones-column appended to V.
  - exp via ScalarE with per-partition bias slope*(j) in exact f32.
  - Causal masking: only the 4 diagonal-intersecting k-blocks per q-tile
    need a 128x128 triangular min-clamp; fully-masked columns are simply
    never computed/streamed.
"""

import sys

if "/opt/trn_rl_repo" not in sys.path:
    sys.path.insert(0, "/opt/trn_rl_repo")

import numpy as np
import ml_dtypes

import concourse.bass as bass
import concourse.bacc as bacc
import concourse.tile as tile
import concourse.mybir as mybir
from concourse import bass_utils

BF16 = mybir.dt.bfloat16
F32 = mybir.dt.float32
NPBF16 = ml_dtypes.bfloat16

B, T, D = 2, 2048, 1024
H, HD = 16, 64
NC = 8
HPC = H // NC          # heads per core = 2
TOK = B * T            # 4096
TPC = TOK // NC        # tokens per core after a2a = 512
NKB = T // 128         # 16 k-blocks per sequence
NQT = T // 512         # 4 q-tiles per sequence
KAUG = HD + 1          # 65: head_dim + 1 aug row

_COMPILED = None


def _build():
    nc = bacc.Bacc("TRN2", target_bir_lowering=False, debug=False, num_devices=NC)

    xT_d = nc.dram_tensor("xT", [D, TOK], BF16, kind="ExternalInput")
    wq_d = nc.dram_tensor("wq", [D, 128], BF16, kind="ExternalInput")
    wk_d = nc.dram_tensor("wk", [D, 128], BF16, kind="ExternalInput")
    wv_d = nc.dram_tensor("wv", [D, 128], BF16, kind="ExternalInput")
    wo_d = nc.dram_tensor("wo", [D, D], BF16, kind="ExternalInput")
    qaug_d = nc.dram_tensor("qaug", [HPC, T], BF16, kind="ExternalInput")
    kbias_d = nc.dram_tensor("kbias", [128, HPC * NKB], F32, kind="ExternalInput")
    cap_d = nc.dram_tensor("cap", [128, 128], F32, kind="ExternalInput")
    ind_d = nc.dram_tensor("ind", [1, 256], BF16, kind="ExternalInput")
    out_d = nc.dram_tensor("out", [TPC, D], F32, kind="ExternalOutput")
    ccin = nc.dram_tensor("ccin", [NC * 128, TPC], BF16, kind="Internal")
    ccout = nc.dram_tensor("ccout", [NC * 128, TPC], BF16, kind="Internal")

    with tile.TileContext(nc) as tc:
        with (
            tc.tile_pool(name="const", bufs=1) as cpool,
            tc.tile_pool(name="work", bufs=1) as wpool,
            tc.tile_pool(name="ps", bufs=4, space="PSUM") as ps,
            tc.tile_pool(name="psot", bufs=3, space="PSUM") as psot,
        ):
            # ---- persistent loads -------------------------------------
            xt = []
            for k in range(8):
                t_ = cpool.tile([128, TOK], BF16, name=f"xt{k}", tag=f"xt{k}")
                nc.sync.dma_start(t_[:], xT_d[128 * k : 128 * (k + 1), :])
                xt.append(t_)
            wq_t = cpool.tile([128, D], BF16, name="wq_t", tag="wq_t")
            wk_t = cpool.tile([128, D], BF16, name="wk_t", tag="wk_t")
            wv_t = cpool.tile([128, D], BF16, name="wv_t", tag="wv_t")
            for k in range(8):
                nc.sync.dma_start(wq_t[:, 128 * k : 128 * (k + 1)], wq_d[128 * k : 128 * (k + 1), :])
                nc.sync.dma_start(wk_t[:, 128 * k : 128 * (k + 1)], wk_d[128 * k : 128 * (k + 1), :])
                nc.sync.dma_start(wv_t[:, 128 * k : 128 * (k + 1)], wv_d[128 * k : 128 * (k + 1), :])
            wo_t = cpool.tile([128, 8 * D], BF16, name="wo_t", tag="wo_t")
            for k in range(8):
                nc.sync.dma_start(wo_t[:, D * k : D * (k + 1)], wo_d[128 * k : 128 * (k + 1), :])
            kbias_t = cpool.tile([128, HPC * NKB], F32, name="kbias_t", tag="kbias_t")
            nc.sync.dma_start(kbias_t[:], kbias_d[:])
            cap_t = cpool.tile([128, 128], F32, name="cap_t", tag="cap_t")
            nc.sync.dma_start(cap_t[:], cap_d[:])
            ind_t = cpool.tile([1, 256], BF16, name="ind_t", tag="ind_t")
            nc.sync.dma_start(ind_t[:], ind_d[:])

            # QTa/KTa: per (b, hl): [65, T]; row 64 is the aug row.
            qta = [[None] * HPC for _ in range(B)]
            kta = [[None] * HPC for _ in range(B)]
            for b in range(B):
                for hl in range(HPC):
                    q_ = cpool.tile([KAUG, T], BF16, name=f"qta{b}{hl}", tag=f"qta{b}{hl}")
                    k_ = cpool.tile([KAUG, T], BF16, name=f"kta{b}{hl}", tag=f"kta{b}{hl}")
                    nc.sync.dma_start(q_[64:65, :], qaug_d[hl : hl + 1, :])
                    nc.vector.memset(k_[64:65, :], 1.0)
                    qta[b][hl] = q_
                    kta[b][hl] = k_
            # V: per b: [128, 16*130]; per k-block: 64 cols head A, ones,
            # 64 cols head B, ones.
            vt = []
            for b in range(B):
                v_ = cpool.tile([128, NKB * 130], BF16, name=f"v{b}", tag=f"v{b}")
                v3 = v_.rearrange("p (t c) -> p t c", c=130)
                nc.vector.memset(v3[:, :, 64], 1.0)
                nc.vector.memset(v3[:, :, 129], 1.0)
                vt.append(v_)

            # ---- phase 1: QKV projections -----------------------------
            for tc8 in range(TOK // 512):
                b, ct = tc8 // 4, (tc8 % 4) * 512
                for w_t, dsts in ((wq_t, qta), (wk_t, kta)):
                    pp = ps.tile([128, 512], F32, name="pp", tag="mm512")
                    for k in range(8):
                        nc.tensor.matmul(
                            pp[:],
                            w_t[:, 128 * k : 128 * (k + 1)],
                            xt[k][:, 512 * tc8 : 512 * (tc8 + 1)],
                            start=(k == 0),
                            stop=(k == 7),
                        )
                    nc.vector.tensor_copy(dsts[b][0][0:64, ct : ct + 512], pp[0:64, :])
                    nc.vector.tensor_copy(dsts[b][1][0:64, ct : ct + 512], pp[64:128, :])
            for g in range(TOK // 128):
                b, kb = g // NKB, g % NKB
                pv = psot.tile([128, 128], F32, name="pv", tag="otv")
                for k in range(8):
                    nc.tensor.matmul(
                        pv[:],
                        xt[k][:, 128 * g : 128 * (g + 1)],
                        wv_t[:, 128 * k : 128 * (k + 1)],
                        start=(k == 0),
                        stop=(k == 7),
                    )
                nc.vector.tensor_copy(vt[b][:, 130 * kb : 130 * kb + 64], pv[:, 0:64])
                nc.vector.tensor_copy(vt[b][:, 130 * kb + 65 : 130 * kb + 129], pv[:, 64:128])

            # ---- phase 2: attention per (b, q-tile) -------------------
            for b in range(B):
                for qt in range(NQT):
                    nkb = 4 * qt + 4
                    ots = []
                    for hl in range(HPC):
                        ot = psot.tile([KAUG, 512], F32, name="ot", tag="otv")
                        for kb in range(nkb):
                            off = max(0, 128 * (kb - 4 * qt))
                            sc = ps.tile([128, 512], F32, name="sc", tag="mm512")
                            nc.tensor.matmul(
                                sc[:, off:512],
                                kta[b][hl][:, 128 * kb : 128 * (kb + 1)],
                                qta[b][hl][:, 512 * qt + off : 512 * (qt + 1)],
                                start=True,
                                stop=True,
                            )
                            if kb >= 4 * qt:
                                nc.vector.tensor_tensor(
                                    sc[:, off : off + 128],
                                    sc[:, off : off + 128],
                                    cap_t[:],
                                    mybir.AluOpType.min,
                                )
                            ex = wpool.tile([128, 512], BF16, name="ex", tag="ex", bufs=4)
                            nc.scalar.activation(
                                ex[:, off:512],
                                sc[:, off:512],
                                mybir.ActivationFunctionType.Exp,
                                bias=kbias_t[:, NKB * hl + kb : NKB * hl + kb + 1],
                                scale=0.125,
                            )
                            nc.tensor.matmul(
                                ot[:, off:512],
                                vt[b][:, 130 * kb + 65 * hl : 130 * kb + 65 * hl + 65],
                                ex[:, off:512],
                                start=(kb == 0),
                                stop=(kb == nkb - 1),
                            )
                        ots.append(ot)
                    recipa = wpool.tile([1, 512], BF16, name="recipa", tag="recipa", bufs=2)
                    recipb = wpool.tile([1, 512], BF16, name="recipb", tag="recipb", bufs=2)
                    with nc.allow_low_precision("softmax recip in bf16: 2^-9 rel ok"):
                        nc.vector.reciprocal(recipa[:], ots[0][64:65, :])
                        nc.vector.reciprocal(recipb[:], ots[1][64:65, :])
                    bc = ps.tile([128, 512], F32, name="bc", tag="mm512")
                    nc.tensor.matmul(bc[:], ind_t[:, 0:128], recipa[:], start=True, stop=False)
                    nc.tensor.matmul(bc[:], ind_t[:, 128:256], recipb[:], start=False, stop=True)
                    bcs = wpool.tile([128, 512], F32, name="bcs", tag="bcs", bufs=2)
                    nc.scalar.copy(bcs[:], bc[:])
                    otn = wpool.tile([128, 512], BF16, name="otn", tag="otn", bufs=3)
                    nc.vector.tensor_tensor(
                        otn[0:64, :], ots[0][0:64, :], bcs[0:64, :], mybir.AluOpType.mult
                    )
                    nc.vector.tensor_tensor(
                        otn[64:128, :], ots[1][0:64, :], bcs[64:128, :], mybir.AluOpType.mult
                    )
                    blk = 4 * b + qt
                    nc.sync.dma_start(ccin[128 * blk : 128 * (blk + 1), :], otn[:])

            # ---- phase 3: all-to-all ----------------------------------
            nc.gpsimd.collective_compute(
                "AllToAll",
                mybir.AluOpType.bypass,
                replica_groups=[list(range(NC))],
                ins=[ccin[:]],
                outs=[ccout[:]],
            )

            # ---- phase 4: output projection ---------------------------
            at = []
            for k in range(8):
                a_ = cpool.tile([128, TPC], BF16, name=f"at{k}", tag=f"at{k}")
                nc.sync.dma_start(a_[:], ccout[128 * k : 128 * (k + 1), :])
                at.append(a_)
            for tb in range(TPC // 128):
                for n in range(D // 512):
                    yp = ps.tile([128, 512], F32, name="yp", tag="mm512")
                    for k in range(8):
                        nc.tensor.matmul(
                            yp[:],
                            at[k][:, 128 * tb : 128 * (tb + 1)],
                            wo_t[:, D * k + 512 * n : D * k + 512 * (n + 1)],
                            start=(k == 0),
                            stop=(k == 7),
                        )
                    ys = wpool.tile([128, 512], F32, name="ys", tag="ys", bufs=2)
                    nc.scalar.copy(ys[:], yp[:])
                    nc.sync.dma_start(
                        out_d[128 * tb : 128 * (tb + 1), 512 * n : 512 * (n + 1)], ys[:]
                    )

    nc.compile()
    return nc


def _host_inputs(x, Wq, Wk, Wv, Wo):
    x = np.asarray(x, dtype=np.float32)
    Wq, Wk, Wv, Wo = (np.asarray(w, dtype=np.float32) for w in (Wq, Wk, Wv, Wo))
    toks = x.reshape(TOK, D)
    xT = np.ascontiguousarray(toks.T).astype(NPBF16)
    wo_t = np.ascontiguousarray(Wo.T).astype(NPBF16)
    base = 2.0 ** (-8.0 / H)

    cap = np.where(
        np.arange(128)[:, None] <= np.arange(128)[None, :], 3.0e38, -1.0e9
    ).astype(np.float32)
    ind = np.zeros((1, 256), dtype=NPBF16)
    ind[0, 0:64] = 1      # head-A indicator: bc rows 0:64 get recipA
    ind[0, 192:256] = 1   # head-B indicator: bc rows 64:128 get recipB
    pos_bf = np.arange(T, dtype=np.float32).astype(NPBF16).astype(np.float32)

    in_maps = []
    for c in range(NC):
        hs = slice(128 * c, 128 * (c + 1))
        qaug = np.zeros((HPC, T), dtype=NPBF16)
        kbias = np.zeros((128, HPC * NKB), dtype=np.float32)
        for hl in range(HPC):
            h = HPC * c + hl
            slope = base ** (h + 1)
            qaug[hl] = (-8.0 * slope * pos_bf).astype(NPBF16)
            for kb in range(NKB):
                kbias[:, NKB * hl + kb] = slope * (128 * kb + np.arange(128))
        in_maps.append(
            {
                "xT": xT,
                "wq": np.ascontiguousarray(Wq[hs, :].T).astype(NPBF16),
                "wk": np.ascontiguousarray(Wk[hs, :].T).astype(NPBF16),
                "wv": np.ascontiguousarray(Wv[hs, :].T).astype(NPBF16),
                "wo": wo_t,
                "qaug": qaug,
                "kbias": kbias,
                "cap": cap,
                "ind": ind,
            }
        )
    return in_maps


def get_compiled():
    global _COMPILED
    if _COMPILED is None:
        _COMPILED = _build()
    return _COMPILED


def run(x, Wq, Wk, Wv, Wo, trace=False, **trace_kwargs):
    nc = get_compiled()
    in_maps = _host_inputs(x, Wq, Wk, Wv, Wo)
    res = bass_utils.run_bass_kernel_spmd(
        nc, in_maps, core_ids=list(range(NC)), trace=trace, **trace_kwargs
    )
    full = np.empty((TOK, D), dtype=np.float32)
    for c in range(NC):
        full[TPC * c : TPC * (c + 1), :] = res.results[c]["out"]
    return full.reshape(B, T, D), res


def kernel(x, Wq, Wk, Wv, Wo):
    out, _ = run(x, Wq, Wk, Wv, Wo)
    return out
